# revision 1
# baseline (speedup 1.0000x reference)
"""Trainium2 Bass kernel: masked squared-error sum, data-parallel on 8 cores.

    total = sum((target - pred)^2  where target != -1.0)

Full inputs: pred, target f32 (4096, 8192).  Row-sharded: core c takes rows
[c*512, (c+1)*512), viewed as (128 partitions, 32768 free) — a free
contiguous reshape.

The host interleaves target and pred per tile into ONE DRAM tensor
x[P, NIT*2*F] so each 128x(2F) tile arrives in a single DMA: TRN2 DVE
instructions only get one semaphore-wait slot (walrus errors with two DMA
waits on a TensorTensor), so both operands must be covered by one DMA sem.

Per tile (t = xt[:, :F], p = xt[:, F:2F]):

    DVE:  diff = t - p                                 (tensor_sub)
    DVE:  md   = (t != -1) * diff                      (scalar_tensor_tensor)
    ACT:  sq   = Square(md), accum_out -> per-partition partial sums

Each tile's 128 partial sums land in one column of a (128, 8) stats tile,
DMA'd to DRAM per core; the host reduces the 8x128x8 partials in float64.
DMA-bound: 32 MiB/core at ~358 GB/s => ~94 us/core roofline.
"""

import numpy as np

_C = 8            # cores
_P = 128          # SBUF partitions
_M, _N = 4096, 8192
_FREE = (_M // _C) * _N // _P   # 32768 free elems per partition per core
_F = 4096                        # tile free size per operand
_NIT = _FREE // _F               # 8 tiles


def _build():
    import concourse.bass as bass
    import concourse.tile as tile
    from concourse import mybir

    nc = bass.Bass()
    x_d = nc.dram_tensor("x", [_P, _NIT * 2 * _F], mybir.dt.float32, kind="ExternalInput")
    out_d = nc.dram_tensor("out", [_P, _NIT], mybir.dt.float32, kind="ExternalOutput")

    # TRN2 compute instructions get ONE semaphore-wait slot (walrus "Too
    # many sync wait commands" otherwise).  Same-engine waits share the
    # engine's own semaphore and merge, so the whole pipeline stays on DVE:
    # each op then carries at most one wait (the DMA RAW for the first
    # consumer, DVE self-waits for the rest).
    with tile.TileContext(nc) as tc:
        half = _NIT // 2
        with (
            tc.tile_pool(name="xp", bufs=3) as xp,
            tc.tile_pool(name="dp", bufs=2) as dp,
            tc.tile_pool(name="mp", bufs=2) as mp,
            tc.tile_pool(name="qp", bufs=2) as qp,
            tc.tile_pool(name="sp", bufs=1) as sp,
        ):
            # Two alternating stats tiles: same-engine WAW at lag 2 is
            # elided by Tile, lag 1 is not — one shared tile would give the
            # ACT a second (self) wait and break the 1-wait limit.
            stats_a = sp.tile([_P, half], mybir.dt.float32, tag="sa")
            stats_b = sp.tile([_P, half], mybir.dt.float32, tag="sb")
            gather = sp.tile([_P, _NIT], mybir.dt.float32, tag="g")
            for i in range(_NIT):
                xt = xp.tile([_P, 2 * _F], mybir.dt.float32, tag="x")
                nc.gpsimd.dma_start(
                    xt[:], x_d[:, i * 2 * _F:(i + 1) * 2 * _F]
                )
                t = xt[:, 0:_F]
                p = xt[:, _F:2 * _F]
                d = dp.tile([_P, _F], mybir.dt.float32, tag="d")
                md = mp.tile([_P, _F], mybir.dt.float32, tag="md")
                sq = qp.tile([_P, 1], mybir.dt.float32, tag="sq")
                nc.vector.tensor_sub(d[:], t, p)
                if i >= 2:
                    # 1-elem sync carrier: absorbs the cross-engine WAR wait
                    # (ACT of iter i-2 still reading this md slot) so the STT
                    # below keeps a single (DVE self) wait.
                    nc.vector.memset(md[:, 0:1], 0.0)
                nc.vector.scalar_tensor_tensor(
                    out=md[:], in0=t, scalar=-1.0, in1=d[:],
                    op0=mybir.AluOpType.not_equal, op1=mybir.AluOpType.mult,
                )
                st = stats_a if i % 2 == 0 else stats_b
                j = i // 2
                nc.scalar.activation(
                    out=sq.broadcast_to(md[:].shape), in_=md[:],
                    func=mybir.ActivationFunctionType.Square,
                    accum_out=st[:, j:j + 1],
                )
            nc.scalar.copy(gather[:, 0:half], stats_a[:])
            nc.scalar.copy(gather[:, half:_NIT], stats_b[:])
            nc.gpsimd.dma_start(out_d[:], gather[:])

    _strip_implied_dma_waits(nc)
    return nc


def _strip_implied_dma_waits(nc):
    """Tile's add_semaphores is not transitively minimal (see 02-tile.md),
    but walrus on this toolchain allows only ONE sem wait per instruction.
    Build the transitive happens-before closure over semaphore events and
    drop waits that are implied by another wait on the same instruction
    (e.g. a slot-reusing DMA's lane-WAW wait is implied by its DVE WAR wait;
    the tail drain's DVE wait is implied by the out-DMA's lane wait)."""
    fn = nc.m.functions[0]
    cum = {}          # sem name -> cumulative update value so far
    facts = {}        # (sem, cum_value) -> dict sem -> min guaranteed value

    def facts_for_wait(name, value):
        # facts guaranteed once `name` reaches >= value: the recorded event
        # with the smallest cum >= value.
        best = None
        for (s, v), f in facts.items():
            if s == name and v >= value and (best is None or v < best[0]):
                best = (v, f)
        return best[1] if best else {}

    def merge(dst, src):
        for k, v in src.items():
            if dst.get(k, 0) < v:
                dst[k] = v

    for blk in fn.blocks:
        for ins in blk.instructions:
            si = ins.sync_info
            if si is None:
                continue
            fin = {}
            for w in si.on_wait:
                if getattr(w, "wait_mode", "") != "sem-ge-imm":
                    continue
                merge(fin, facts_for_wait(w.ant_name, w.wait_value))
                merge(fin, {w.ant_name: w.wait_value})
            for u in si.on_update:
                prev = cum.get(u.ant_name, 0)
                new = prev + (u.update_value or 0)
                cum[u.ant_name] = new
                f = dict(fin)
                # same-sem monotonicity: inherits the previous value's facts
                merge(f, facts.get((u.ant_name, prev), {}))
                if prev:
                    merge(f, {u.ant_name: prev})
                facts[(u.ant_name, new)] = f

    for blk in fn.blocks:
        for ins in blk.instructions:
            si = ins.sync_info
            if si is None or len(si.on_wait) <= 1:
                continue
            ws = list(si.on_wait)
            if any(getattr(w, "wait_mode", "") != "sem-ge-imm" for w in ws):
                continue
            kept = []
            for i, w in enumerate(ws):
                implied = False
                for j, w2 in enumerate(ws):
                    if i == j:
                        continue
                    f2 = facts_for_wait(w2.ant_name, w2.wait_value)
                    if f2.get(w.ant_name, 0) >= w.wait_value:
                        # mutual implication: keep the lower-indexed one
                        own = facts_for_wait(w.ant_name, w.wait_value)
                        mutual = own.get(w2.ant_name, 0) >= w2.wait_value
                        if not mutual or j < i:
                            implied = True
                            break
                if not implied:
                    kept.append(w)
            if len(kept) != len(ws):
                si.on_wait = kept
                ins.sync_info = si


def _shard(pred, target):
    pred_r = np.ascontiguousarray(pred, dtype=np.float32).reshape(_C, _P, _NIT, _F)
    targ_r = np.ascontiguousarray(target, dtype=np.float32).reshape(_C, _P, _NIT, _F)
    x = np.empty((_C, _P, _NIT, 2, _F), dtype=np.float32)
    x[:, :, :, 0, :] = targ_r
    x[:, :, :, 1, :] = pred_r
    return [{"x": x[c].reshape(_P, _NIT * 2 * _F)} for c in range(_C)]


def run(pred, target, **spmd_kwargs):
    """Build + run on all 8 cores; returns (scalar_output, BassKernelResults)."""
    from concourse.bass_utils import run_bass_kernel_spmd

    nc = _build()
    res = run_bass_kernel_spmd(
        nc, _shard(pred, target), core_ids=list(range(_C)), **spmd_kwargs
    )
    total = 0.0
    for c in range(_C):
        total += res.results[c]["out"].astype(np.float64).sum()
    return np.array(total, dtype=np.float32), res


def kernel(pred: np.ndarray, target: np.ndarray) -> np.ndarray:
    out, _ = run(pred, target)
    return out

